# revision 32
# baseline (speedup 1.0000x reference)
"""Trainium2 Bass kernel for nn_CSAB2 (cross-set attention block, 8 cores).

Sharding: zero-collective. 8 cores = 4 batches x 2 output sides (x / y).
Each core computes one full output O_x[b] or O_y[b] (1024, 1024) from its
own sequence A, the other sequence C, and the 5 weight matrices its side
needs (Q projection, own K/V, other K/V) plus the two FC halves.

Per-core math (uniform SPMD program):
  Qt  = (Wq^T A^T) + bq          feature-major [f, i]
  K1t = (Wk1^T A^T) + bk1        K2t = (Wk2^T C^T) + bk2
  V1  = A Wv1 + bv1 (token-major, with an appended ones column)
  V2  = C Wv2 + bv2
  For attn in {own(K1,V1), oth(K2,V2)}, per head h (DH=128):
    St = K_h^T tiles . Q_h  -> P = exp(St/32)   [j, i] layout
    PV[i, 0:128|128] = P^T-tiles . [V_h | 1]    (denominator for free)
    Z[i, h*128:] = PV[:, :128] * (1/PV[:, 128]) + Q_tok[i, h*128:]
  L_own = LN(Z_own), L_oth = LN(Z_oth)  (g0/b0 folded into W1/W2/fcb on host)
  U = L_own W1 + L_oth W2 + fcb;  U = relu(U);  O = LN(U) * g1 + b1

All matmul operands fp16 (measured HW rel-err 3e-4); accumulation,
softmax, layernorm in fp32.
"""

import sys

sys.path.insert(0, "/opt/trn_rl_repo")

import numpy as np

import concourse.bass as bass
import concourse.tile as tile
from concourse import bacc, mybir
from concourse.bass_utils import run_bass_kernel_spmd

N = 1024  # tokens per sequence
D = 1024  # model dim
H = 8  # heads
DH = 128  # head dim
P = 128  # partitions
NT = N // P  # 8 token tiles
FT = D // P  # 8 feature tiles
EPS = 1e-5
F16 = mybir.dt.float16
F32 = mybir.dt.float32
SCALE = 1.0 / 32.0  # 1/sqrt(D)

_CACHED = {}


def _bcast_ap(vec_ap, cols):
    """[cols]-element DRAM vector -> [128, cols] partition-broadcast AP."""
    return bass.AP(
        tensor=vec_ap.tensor, offset=vec_ap.offset, ap=[[0, P], [1, cols]]
    )


def _ln_normalize(nc, pool_small, zc, out_tile, stats_dt=F32):
    """LayerNorm zc [128, 1024] f32 along free dim -> out_tile (no affine)."""
    stats = pool_small.tile([P, 2, 6], F32, tag="ln_stats")
    for sg in range(2):
        nc.vector.bn_stats(out=stats[:, sg, :], in_=zc[:, sg * 512 : (sg + 1) * 512])
    mv = pool_small.tile([P, 2], F32, tag="ln_mv")
    nc.vector.bn_aggr(out=mv[:], in_=stats[:])
    std = pool_small.tile([P, 1], F32, tag="ln_std")
    nc.scalar.activation(
        out=std[:],
        in_=mv[:, 1:2],
        func=mybir.ActivationFunctionType.Sqrt,
        bias=_CACHED["eps_tile"][:],
        scale=1.0,
    )
    rstd = pool_small.tile([P, 1], F32, tag="ln_rstd")
    nc.vector.reciprocal(out=rstd[:], in_=std[:])
    nc.vector.tensor_scalar(
        out=out_tile,
        in0=zc[:],
        scalar1=mv[:, 0:1],
        scalar2=rstd[:],
        op0=mybir.AluOpType.subtract,
        op1=mybir.AluOpType.mult,
    )


def _build():
    """Build + compile the per-core SPMD program. Returns compiled Bacc."""
    nc = bacc.Bacc(None, target_bir_lowering=False, debug=False)

    dram = {}
    for nm in ("a", "cc", "wq", "wk1", "wk2", "wv1", "wv2", "w1", "w2"):
        dram[nm] = nc.dram_tensor(nm, (D, D), F16, kind="ExternalInput")
    for nm in ("bq", "bk1", "bk2", "bv1", "bv2", "fcb", "g1v", "b1v"):
        dram[nm] = nc.dram_tensor(nm, (D,), F32, kind="ExternalInput")
    o_dram = nc.dram_tensor("o", (N, D), F32, kind="ExternalOutput")

    with tile.TileContext(nc) as tc:
        import contextlib

        ctx = contextlib.ExitStack()
        with ctx:
            const = ctx.enter_context(tc.tile_pool(name="const", bufs=1))
            small = ctx.enter_context(tc.tile_pool(name="small", bufs=6))
            eps_tile = const.tile([P, 1], F32, tag="eps")
            nc.vector.memset(eps_tile[:], EPS)
            _CACHED["eps_tile"] = eps_tile

            # per-partition bias layouts: [p, ft] with element = vec[ft*128+p]
            bias_sb = {}
            for nm in ("bq", "bk1", "bk2"):
                t = const.tile([P, FT], F32, tag=f"{nm}_sb")
                nc.sync.dma_start(t[:], dram[nm][:].rearrange("(t p) -> p t", p=P))
                bias_sb[nm] = t

            persist = ctx.enter_context(tc.tile_pool(name="persist", bufs=1))
            lt = {
                "own": persist.tile([P, FT, N], F16, tag="lto", name="lt_own"),
                "oth": persist.tile([P, FT, N], F16, tag="ltx", name="lt_oth"),
            }
            # attention-phase data; freed before the FC phase
            attn_data = tc.alloc_tile_pool(name="attn_data", bufs=1)
            qt = [attn_data.tile([P, N], F16, tag=f"qt{i}", name=f"qt{i}") for i in range(FT)]
            k1t = [attn_data.tile([P, N], F16, tag=f"k1t{i}", name=f"k1t{i}") for i in range(FT)]
            k2t = [attn_data.tile([P, N], F16, tag=f"k2t{i}", name=f"k2t{i}") for i in range(FT)]
            qtok_all = attn_data.tile([P, NT, D], F16, tag="qtok", name="qtok_all")
            v1 = [attn_data.tile([P, H, DH + 1], F16, tag=f"v1_{j}", name=f"v1_{j}") for j in range(NT)]
            v2 = [attn_data.tile([P, H, DH + 1], F16, tag=f"v2_{j}", name=f"v2_{j}") for j in range(NT)]

            def transpose_in(src_dram, dst_all):
                """One transposing DMA: dst_all[p, dt, i] = src[i, dt*128+p]."""
                nc.scalar.dma_start_transpose(dst_all[:], src_dram[:])

            def project_fmajor(specs, src_t, bufs=4):
                """For each (w_dram, bias_col, out_tiles) in specs, emit
                out[f,i] = sum_d W[d,f] * srcT[d,i] + bias, interleaved per
                f-tile so downstream consumers of early f-tiles unblock
                sooner."""
                with (
                    tc.tile_pool(name="wpan", bufs=6) as wpan,
                    tc.tile_pool(name="psum_proj", bufs=bufs, space="PSUM") as psum_p,
                ):
                    for ft in range(FT):
                        for w_dram, bias_col, out_tiles in specs:
                            wp = wpan.tile([P, FT, P], F16, tag="wp")
                            nc.sync.dma_start(
                                wp[:],
                                w_dram[:, ft * P : (ft + 1) * P].rearrange(
                                    "(dt p) f -> p dt f", p=P
                                ),
                            )
                            ps = psum_p.tile([P, D], F32, tag="ps_proj")
                            for ic in range(2):
                                for d in range(FT):
                                    nc.tensor.matmul(
                                        ps[:, ic * 512 : (ic + 1) * 512],
                                        wp[:, d, :],
                                        src_t[d][:, ic * 512 : (ic + 1) * 512],
                                        start=(d == 0),
                                        stop=(d == FT - 1),
                                    )
                            nc.vector.tensor_scalar_add(
                                out_tiles[ft][:], ps[:], bias_col[:, ft : ft + 1]
                            )

            def project_tmajor(w_dram, src_t, bias_vec_dram, out_tiles, bufs=3):
                """out[j,f] = sum_d srcT[d,j]^T W[d,f] + bias (free-dim) into
                [P, H, DH+1] tiles; also set the ones column."""
                with (
                    tc.tile_pool(name="vb_pool", bufs=1) as vb_pool,
                    tc.tile_pool(name="wvpan", bufs=1) as wvpan,
                    tc.tile_pool(name="psum_projv", bufs=bufs, space="PSUM") as psum_p,
                ):
                    bv_bc = vb_pool.tile([P, D], F32, tag="bv_bc")
                    nc.sync.dma_start(bv_bc[:], _bcast_ap(bias_vec_dram[:], D))
                    wps = []
                    for fc in range(2):
                        wp = wvpan.tile([P, FT, 512], F16, tag=f"wvp{fc}", name=f"wvp{fc}")
                        nc.sync.dma_start(
                            wp[:],
                            w_dram[:, fc * 512 : (fc + 1) * 512].rearrange(
                                "(dt p) f -> p dt f", p=P
                            ),
                        )
                        wps.append(wp)
                    for j in range(NT):
                        ps = psum_p.tile([P, D], F32, tag="ps_proj")
                        for fc in range(2):
                            for d in range(FT):
                                nc.tensor.matmul(
                                    ps[:, fc * 512 : (fc + 1) * 512],
                                    src_t[d][:, j * P : (j + 1) * P],
                                    wps[fc][:, d, :],
                                    start=(d == 0),
                                    stop=(d == FT - 1),
                                )
                        nc.vector.tensor_add(
                            out_tiles[j][:, :, 0:DH],
                            ps[:].rearrange("p (h f) -> p h f", f=DH),
                            bv_bc[:].rearrange("p (h f) -> p h f", f=DH),
                        )
                for j in range(NT):
                    nc.vector.memset(out_tiles[j][:, :, DH : DH + 1], 1.0)

            # ---- phase A: own sequence -> At -> Qt, K1t, V1, Qtok ----
            with tc.tile_pool(name="at_pool", bufs=1) as at_pool:
                at_all = at_pool.tile([P, FT, N], F16, tag="at_all", name="at_all")
                at = [at_all[:, j, :] for j in range(FT)]
                transpose_in(dram["a"], at_all)
                project_fmajor(
                    [
                        (dram["wq"], bias_sb["bq"], qt),
                        (dram["wk1"], bias_sb["bk1"], k1t),
                    ],
                    at,
                )
                project_tmajor(dram["wv1"], at, dram["bv1"], v1)
            # Q token-major for the residual: qtok_all[p, it, f] = Q[it*128+p, f]
            for ft in range(FT):
                nc.scalar.dma_start_transpose(
                    qtok_all[:, :, ft * P : (ft + 1) * P], qt[ft][:]
                )

            # ---- phase B: other sequence -> Ct -> K2t, V2 ----
            with tc.tile_pool(name="ct_pool", bufs=1) as ct_pool:
                ct_all = ct_pool.tile([P, FT, N], F16, tag="ct_all", name="ct_all")
                ct = [ct_all[:, j, :] for j in range(FT)]
                transpose_in(dram["cc"], ct_all)
                project_fmajor([(dram["wk2"], bias_sb["bk2"], k2t)], ct)
                project_tmajor(dram["wv2"], ct, dram["bv2"], v2)

            # ---- attention block ----
            attn_pools = (
                tc.alloc_tile_pool(name="p_pool", bufs=8),
                tc.alloc_tile_pool(name="z_pool", bufs=6),
                tc.alloc_tile_pool(name="psum_s", bufs=3, space="PSUM"),
                tc.alloc_tile_pool(name="psum_pv", bufs=2, space="PSUM"),
            )
            p_pool, z_pool, psum_s, psum_pv = attn_pools

            def attn_block(key, kt_t, v_t):
                    for ic in range(2):
                        zc = [z_pool.tile([P, D], F32, tag="zc", name="zc") for _ in range(4)]
                        for h in range(H):
                            # P stored as 4 pair-tiles, each holding 2 j-tiles
                            p_t = [
                                p_pool.tile([P, 2, 512], F16, tag="p_t", name="p_t")
                                for _ in range(NT // 2)
                            ]
                            for jp in range(NT // 2):
                                sps = psum_s.tile([P, 2, 512], F32, tag="sps")
                                for half in range(2):
                                    nc.tensor.matmul(
                                        sps[:, half, :],
                                        kt_t[h][
                                            :, (2 * jp + half) * P : (2 * jp + half + 1) * P
                                        ],
                                        qt[h][:, ic * 512 : (ic + 1) * 512],
                                        start=True,
                                        stop=True,
                                    )
                                nc.scalar.activation(
                                    out=p_t[jp][:],
                                    in_=sps[:],
                                    func=mybir.ActivationFunctionType.Exp,
                                    scale=SCALE,
                                )
                            for ip in range(2):
                                # two i-tiles share one PSUM bank: [128, 2, 129]
                                pvp = psum_pv.tile([P, 2, DH + 1], F32, tag="pvp")
                                for half in range(2):
                                    il = 2 * ip + half
                                    for jt in range(NT):
                                        nc.tensor.matmul(
                                            pvp[:, half, :],
                                            p_t[jt // 2][:, jt % 2, il * P : (il + 1) * P],
                                            v_t[jt][:, h, :],
                                            start=(jt == 0),
                                            stop=(jt == NT - 1),
                                        )
                                rcp = small.tile([P, 2], F32, tag="rcp")
                                nc.vector.reciprocal(rcp[:], pvp[:, :, DH])
                                for half in range(2):
                                    il = 2 * ip + half
                                    it = ic * 4 + il
                                    nc.vector.scalar_tensor_tensor(
                                        out=zc[il][:, h * DH : (h + 1) * DH],
                                        in0=pvp[:, half, 0:DH],
                                        scalar=rcp[:, half : half + 1],
                                        in1=qtok_all[:, it, h * DH : (h + 1) * DH],
                                        op0=mybir.AluOpType.mult,
                                        op1=mybir.AluOpType.add,
                                    )
                        # LN per completed token tile, then transpose into Lt
                        for il in range(4):
                            it = ic * 4 + il
                            ltok = small.tile([P, D], F16, tag="ltok")
                            _ln_normalize(nc, small, zc[il], ltok[:])
                            nc.scalar.dma_start_transpose(
                                lt[key][:, :, it * P : (it + 1) * P], ltok[:]
                            )

            attn_block("own", k1t, v1)
            attn_block("oth", k2t, v2)

            for pool in reversed(attn_pools):
                pool.release()

            # ---- FC + relu + final LN ----
            attn_data.release()  # free qt/kt/v/qtok before FC allocations

            with (
                tc.tile_pool(name="fc_const", bufs=1) as fc_const,
                tc.tile_pool(name="u_pool", bufs=3) as u_pool,
                tc.tile_pool(name="wf_pool", bufs=1) as wf_pool,
                tc.tile_pool(name="psum_fc", bufs=3, space="PSUM") as psum_fc,
                tc.tile_pool(name="out_pool", bufs=3) as out_pool,
            ):
                fcb_bc = fc_const.tile([P, D], F32, tag="fcb_bc")
                nc.sync.dma_start(fcb_bc[:], _bcast_ap(dram["fcb"][:], D))
                g1_bc = fc_const.tile([P, D], F32, tag="g1_bc")
                nc.sync.dma_start(g1_bc[:], _bcast_ap(dram["g1v"][:], D))
                b1_bc = fc_const.tile([P, D], F32, tag="b1_bc")
                nc.sync.dma_start(b1_bc[:], _bcast_ap(dram["b1v"][:], D))
                wps = {}
                for oc in range(2):
                    for nm in ("w1", "w2"):
                        wp = wf_pool.tile(
                            [P, FT, 512], F16, tag=f"wf_{nm}{oc}", name=f"wf_{nm}{oc}"
                        )
                        nc.sync.dma_start(
                            wp[:],
                            dram[nm][:, oc * 512 : (oc + 1) * 512].rearrange(
                                "(dt p) f -> p dt f", p=P
                            ),
                        )
                        wps[(nm, oc)] = wp
                # it-outer: finish each token tile (both oc halves + relu + LN
                # + store) before moving on -> short serial tail
                for it in range(NT):
                    ut = u_pool.tile([P, D], F32, tag="ut", name="ut")
                    for oc in range(2):
                        fps = psum_fc.tile([P, 512], F32, tag="fps")
                        for kt in range(FT):
                            nc.tensor.matmul(
                                fps[:],
                                lt["own"][:, kt, it * P : (it + 1) * P],
                                wps[("w1", oc)][:, kt, :],
                                start=(kt == 0),
                                stop=False,
                            )
                        for kt in range(FT):
                            nc.tensor.matmul(
                                fps[:],
                                lt["oth"][:, kt, it * P : (it + 1) * P],
                                wps[("w2", oc)][:, kt, :],
                                start=False,
                                stop=(kt == FT - 1),
                            )
                        nc.vector.tensor_add(
                            ut[:, oc * 512 : (oc + 1) * 512],
                            fps[:],
                            fcb_bc[:, oc * 512 : (oc + 1) * 512],
                        )
                    nc.vector.tensor_scalar_max(ut[:], ut[:], 0.0)
                    ot = out_pool.tile([P, D], F32, tag="ot")
                    _ln_normalize(nc, small, ut[:], ot[:])
                    nc.vector.tensor_mul(ot[:], ot[:], g1_bc[:])
                    nc.vector.tensor_add(ot[:], ot[:], b1_bc[:])
                    nc.sync.dma_start(o_dram[it * P : (it + 1) * P, :], ot[:])

    nc.compile()
    return nc


def build_in_maps(X, Y, Wqx, bqx, Wkx, bkx, Wvx, bvx, Wqy, bqy, Wky, bky,
                  Wvy, bvy, WX, bX, WY, bY, g0, b0, g1, b1):
    f = lambda t: np.asarray(t, dtype=np.float32)
    h = lambda t: np.ascontiguousarray(np.asarray(t, dtype=np.float32).astype(np.float16))
    X, Y = f(X), f(Y)
    g0d, b0d = f(g0).astype(np.float64), f(b0).astype(np.float64)
    g1f, b1f = f(g1), f(b1)

    sides = {}
    for side, W, bo in (("x", f(WX), f(bX)), ("y", f(WY), f(bY))):
        Wtop = W[:D].astype(np.float64)
        Wbot = W[D:].astype(np.float64)
        fcb = (b0d @ Wtop + b0d @ Wbot + bo.astype(np.float64)).astype(np.float32)
        w_top_folded = (g0d[:, None] * Wtop).astype(np.float32)
        w_bot_folded = (g0d[:, None] * Wbot).astype(np.float32)
        if side == "x":
            # concat order [own=O_xx, oth=O_xy]
            w_own, w_oth = w_top_folded, w_bot_folded
        else:
            # concat order [oth=O_yx, own=O_yy]
            w_own, w_oth = w_bot_folded, w_top_folded
        sides[side] = dict(w1=h(w_own), w2=h(w_oth), fcb=fcb)

    wx = dict(wq=h(Wqx), bq=f(bqx), wk=h(Wkx), bk=f(bkx), wv=h(Wvx), bv=f(bvx))
    wy = dict(wq=h(Wqy), bq=f(bqy), wk=h(Wky), bk=f(bky), wv=h(Wvy), bv=f(bvy))

    in_maps = []
    for core in range(8):
        b = core // 2
        side = "x" if core % 2 == 0 else "y"
        own, oth = (wx, wy) if side == "x" else (wy, wx)
        a_seq = X[b] if side == "x" else Y[b]
        c_seq = Y[b] if side == "x" else X[b]
        in_maps.append({
            "a": h(a_seq), "cc": h(c_seq),
            "wq": own["wq"], "bq": own["bq"],
            "wk1": own["wk"], "bk1": own["bk"],
            "wv1": own["wv"], "bv1": own["bv"],
            "wk2": oth["wk"], "bk2": oth["bk"],
            "wv2": oth["wv"], "bv2": oth["bv"],
            "w1": sides[side]["w1"], "w2": sides[side]["w2"],
            "fcb": sides[side]["fcb"],
            "g1v": g1f, "b1v": b1f,
        })
    return in_maps


def kernel(**inputs):
    if "nc" not in _CACHED:
        _CACHED["nc"] = _build()
    nc = _CACHED["nc"]

    in_maps = build_in_maps(**inputs)
    res = run_bass_kernel_spmd(nc, in_maps, list(range(8)))
    _CACHED["last_result"] = res

    B = np.asarray(inputs["X"]).shape[0]
    O_x = np.stack([res.results[2 * b]["o"] for b in range(B)])
    O_y = np.stack([res.results[2 * b + 1]["o"] for b in range(B)])
    return O_x, O_y
